# revision 16
# baseline (speedup 1.0000x reference)
"""Trainium2 Bass kernel for nn_Attention (B=4,S=1024,NX=1024,H=16).

Sharding (8 cores): data-parallel over batch (4) x tensor-parallel over
head-groups (2 groups of 8 heads), Megatron-style. Each core computes
QKV projection for its 8 heads on its batch, causal attention, and the
partial c_proj contribution; the 2-way all-reduce after c_proj is done
as a pair-sum during the host-side gather (output of each core is a
distinct partial), which is mathematically the Megatron all-reduce.

Device kernel (per core, fully unrolled Tile program):
  - inputs: xT = x[b].T [NX,S], wqkv [NX,1536] (q|k|v cols of this
    head group), wproj [512,NX] (rows of this head group), battn [1536]
  - qT,kT ([m,s] layout) via matmul(lhsT=wqkv, rhs=xT) + bias
  - v ([s,m] layout) via matmul(lhsT=xT, rhs=w_v) + bias (broadcast)
  - scores transposed S^T[sk,sq] = kT.T @ qT per head, exp on ScalarE
    (scale=1/8 folded in, no max-subtraction needed: |logits|<~6),
    causal handled structurally: fully-masked blocks never computed,
    diagonal blocks multiplied by a triangular 0/1 mask built on-device
  - av: aT[d,sq] = v65.T @ P^T with a ones-column in v65 producing the
    softmax denominator as row 64 of the PSUM tile (partition reduce
    done by the PE); normalize with reciprocal broadcast
  - partial out = aT.T @ wproj accumulated in PSUM, DMA'd straight out
All matmuls run as float32r (full PE rate at N=512, near-fp32 accuracy).
"""

import sys

sys.path.insert(0, "/opt/trn_rl_repo")

import numpy as np

import concourse.bass as bass
import concourse.mybir as mybir
import concourse.tile as tile
from concourse import bacc

B, S, NX, H = 4, 1024, 1024, 16
D = NX // H            # 64
HPC = 8                # heads per core (2 groups)
M_LOC = HPC * D        # 512 columns of q/k/v per core
P = 128
NK = NX // P           # 8 contraction chunks
SC = S // P            # 8 sequence chunks of 128
F32 = mybir.dt.float32
F32R = mybir.dt.float32r
AF = mybir.ActivationFunctionType
ALU = mybir.AluOpType


def _r(ap):
    """float32r view of an fp32 AP (full-rate PE matmul)."""
    return ap


def build_nc():
    nc = bacc.Bacc("TRN2", target_bir_lowering=False, debug=False, num_devices=8)
    # f32r = fp32 storage, reduced-precision full-rate PE matmul. Inputs
    # feeding matmuls are declared f32r end-to-end (DMA preserves dtype,
    # satisfying the BIR verifier's rounding rule); np side sees float32.
    xT = nc.dram_tensor("xT", [NX, S], F32R, kind="ExternalInput")
    wqkv = nc.dram_tensor("wqkv", [NX, 3 * M_LOC], F32R, kind="ExternalInput")
    wproj = nc.dram_tensor("wproj", [M_LOC, NX], F32R, kind="ExternalInput")
    battn = nc.dram_tensor("battn", [3 * M_LOC], F32, kind="ExternalInput")
    out_part = nc.dram_tensor("out_part", [S, NX], F32, kind="ExternalOutput")
    kT_out = nc.dram_tensor("kT_out", [M_LOC, S], F32R, kind="ExternalOutput")
    v_out = nc.dram_tensor("v_out", [S, M_LOC], F32R, kind="ExternalOutput")

    xT_t = xT.rearrange("(nk p) s -> p nk s", p=P)
    wqkv_t = wqkv.rearrange("(nk p) m -> p nk m", p=P)
    wproj_t = wproj.rearrange("(mk p) n -> p mk n", p=P)
    battn_t = battn.rearrange("(mc p) -> p mc", p=P)
    out_t = out_part.rearrange("(sc p) n -> p sc n", p=P)
    kT_t = kT_out.rearrange("(kc p) s -> p kc s", p=P)
    v_t = v_out.rearrange("(sc p) m -> p sc m", p=P)

    with tile.TileContext(nc) as tc:
        with (
            tc.tile_pool(name="const", bufs=1) as const,
            tc.tile_pool(name="wq", bufs=3) as wq_pool,
            tc.tile_pool(name="vb", bufs=3) as vb_pool,
            tc.tile_pool(name="pt", bufs=2) as pt_pool,
            tc.tile_pool(name="small", bufs=3) as small,
            tc.tile_pool(name="ps_mm", bufs=2, space="PSUM") as ps_mm,
            tc.tile_pool(name="ps_pt", bufs=2, space="PSUM") as ps_pt,
            tc.tile_pool(name="ps_a", bufs=2, space="PSUM") as ps_a,
        ):
            # ---- resident tiles ----
            xT_sb = const.tile([P, NK, S], F32R)         # x[b].T
            qkT_sb = const.tile([P, 8, S], F32R)         # qT (mc 0-3) | kT (mc 4-7)
            wv_sb = const.tile([P, NK, M_LOC], F32R)     # v columns of wqkv
            wproj_sb = const.tile([P, 4, NX], F32R)
            v65_sb = const.tile([P, SC, HPC, D + 1], F32R)  # v + ones col
            aT_sb = const.tile([P, 4, S], F32R)          # normalized attn out ^T
            battn_sb = const.tile([P, 12], F32)
            bv_row = const.tile([1, M_LOC], F32)
            bv_rep = const.tile([P, M_LOC], F32)
            tri_sb = const.tile([P, P], F32)             # keep where col>=row

            nc.sync.dma_start(xT_sb[:], xT_t)
            nc.sync.dma_start(wv_sb[:], wqkv_t[:, :, 2 * M_LOC : 3 * M_LOC])
            nc.sync.dma_start(wproj_sb[:], wproj_t)
            nc.sync.dma_start(battn_sb[:], battn_t)
            nc.sync.dma_start(
                bv_row[:], battn[2 * M_LOC : 3 * M_LOC].rearrange("(a m) -> a m", a=1)
            )
            nc.gpsimd.partition_broadcast(bv_rep[:], bv_row[:])

            nc.gpsimd.memset(tri_sb[:], 1.0)
            nc.gpsimd.affine_select(
                out=tri_sb[:],
                in_=tri_sb[:],
                pattern=[[1, P]],
                compare_op=ALU.is_ge,
                fill=0.0,
                base=0,
                channel_multiplier=-1,
            )


            # ---- phase B1: qT / kT  (out[m_chunk, s] = wqkv.T @ xT) ----
            for mc in range(8):
                wq_sb = wq_pool.tile([P, NK, P], F32R, tag="wq")
                nc.sync.dma_start(wq_sb[:], wqkv_t[:, :, mc * P : (mc + 1) * P])
                for sc in range(2):
                    ps = ps_mm.tile([P, 512], F32, tag="mm")
                    for nk in range(NK):
                        nc.tensor.matmul(
                            ps[:],
                            _r(wq_sb[:, nk, :]),
                            _r(xT_sb[:, nk, sc * 512 : (sc + 1) * 512]),
                            start=(nk == 0),
                            stop=(nk == NK - 1),
                        )
                    nc.vector.tensor_scalar_add(
                        qkT_sb[:, mc, sc * 512 : (sc + 1) * 512],
                        ps[:],
                        battn_sb[:, mc : mc + 1],
                    )
            for kc in range(4):
                nc.sync.dma_start(kT_t[:, kc], qkT_sb[:, 4 + kc, :])

            # ---- phase B2: v natural (out[s_chunk, m] = xT.T @ w_v) ----
            for sc in range(SC):
                ps = ps_mm.tile([P, 512], F32, tag="mm")
                for nk in range(NK):
                    nc.tensor.matmul(
                        ps[:],
                        _r(xT_sb[:, nk, sc * P : (sc + 1) * P]),
                        _r(wv_sb[:, nk, :]),
                        start=(nk == 0),
                        stop=(nk == NK - 1),
                    )
                vb_sb = vb_pool.tile([P, M_LOC], F32R, tag="vb")
                nc.vector.tensor_tensor(vb_sb[:], ps[:], bv_rep[:], ALU.add)
                nc.sync.dma_start(v_t[:, sc], vb_sb[:])
                for h in range(HPC):
                    nc.vector.tensor_copy(
                        out=v65_sb[:, sc, h, 0:D], in_=vb_sb[:, h * D : (h + 1) * D]
                    )
                # ones column (softmax denominator row): 0*in + 1 via ACT
                nc.scalar.activation(
                    v65_sb[:, sc, :, D], vb_sb[:, 0:HPC], AF.Copy, scale=0.0, bias=1.0
                )

            # ---- phase C: attention per (sq half, head) ----
            for sqh in range(2):
                L = (sqh + 1) * 512          # causal kv length for this q half
                nsk = L // P
                for h in range(HPC):
                    po = (h % 2) * 64
                    qmc = h // 2
                    kmc = 4 + h // 2
                    pt_sb = pt_pool.tile([P, SC, 512], F32R, tag="pt")
                    for skc in range(nsk):
                        j_tri = skc - sqh * 4  # diag block index within this half
                        j0 = max(0, j_tri)
                        ncols = 512 - j0 * P
                        ps = ps_pt.tile([P, 512], F32, tag="pt")
                        nc.tensor.matmul(
                            ps[:, j0 * P : 512],
                            _r(qkT_sb[po : po + 64, kmc, skc * P : (skc + 1) * P]),
                            _r(
                                qkT_sb[
                                    po : po + 64,
                                    qmc,
                                    sqh * 512 + j0 * P : (sqh + 1) * 512,
                                ]
                            ),
                            start=True,
                            stop=True,
                        )
                        nc.scalar.activation(
                            pt_sb[:, skc, j0 * P : 512],
                            ps[:, j0 * P : 512],
                            AF.Exp,
                            scale=float(1.0 / np.sqrt(D)),
                        )
                        if j_tri >= 0:
                            nc.vector.tensor_tensor(
                                pt_sb[:, skc, j_tri * P : (j_tri + 1) * P],
                                pt_sb[:, skc, j_tri * P : (j_tri + 1) * P],
                                tri_sb[:],
                                ALU.mult,
                            )
                    # av: aT[d,sq] (+ denominator in row 64 via ones column);
                    # each k chunk only touches its causally-valid sq columns
                    pa = ps_a.tile([D + 1, 512], F32, tag="pa")
                    for skc in range(nsk):
                        j0 = max(0, skc - sqh * 4)
                        nc.tensor.matmul(
                            pa[:, j0 * P : 512],
                            _r(v65_sb[:, skc, h, :]),
                            _r(pt_sb[:, skc, j0 * P : 512]),
                            start=(skc == 0),
                            stop=(skc == nsk - 1),
                        )
                    rinv = small.tile([1, 512], F32, tag="rinv")
                    nc.vector.reciprocal(rinv[:], pa[D : D + 1, :])
                    rrep = small.tile([D, 512], F32, tag="rrep")
                    nc.gpsimd.partition_broadcast(rrep[:], rinv[:])
                    a64 = small.tile([D, 512], F32R, tag="a64")
                    nc.vector.tensor_tensor(a64[:], pa[0:D, :], rrep[:], ALU.mult)
                    # place head rows at partitions po..po+64 of aT chunk h//2
                    nc.sync.dma_start(
                        aT_sb[po : po + 64, h // 2, sqh * 512 : (sqh + 1) * 512],
                        a64[:],
                    )

            # ---- phase D: partial c_proj ----
            for sc in range(SC):
                for nch in range(2):
                    ps = ps_mm.tile([P, 512], F32, tag="mm")
                    for mk in range(4):
                        nc.tensor.matmul(
                            ps[:],
                            _r(aT_sb[:, mk, sc * P : (sc + 1) * P]),
                            _r(wproj_sb[:, mk, nch * 512 : (nch + 1) * 512]),
                            start=(mk == 0),
                            stop=(mk == 3),
                        )
                    o_sb = vb_pool.tile([P, 512], F32, tag="osb")
                    nc.vector.tensor_copy(out=o_sb[:], in_=ps[:])
                    nc.sync.dma_start(out_t[:, sc, nch * 512 : (nch + 1) * 512], o_sb[:])

    nc.compile()
    return nc


_NC = None


def kernel(**inputs):
    global _NC
    from concourse.bass_utils import run_bass_kernel_spmd

    x = np.asarray(inputs["x"], dtype=np.float32)
    w_attn = np.asarray(inputs["w_attn"], dtype=np.float32)
    b_attn = np.asarray(inputs["b_attn"], dtype=np.float32)
    w_proj = np.asarray(inputs["w_proj"], dtype=np.float32)
    b_proj = np.asarray(inputs["b_proj"], dtype=np.float32)

    if _NC is None:
        _NC = build_nc()
    nc = _NC

    in_maps = []
    for c in range(8):
        b, g = c // 2, c % 2
        cols = slice(g * M_LOC, (g + 1) * M_LOC)
        wqkv = np.concatenate(
            [
                w_attn[:, cols],
                w_attn[:, NX + g * M_LOC : NX + (g + 1) * M_LOC],
                w_attn[:, 2 * NX + g * M_LOC : 2 * NX + (g + 1) * M_LOC],
            ],
            axis=1,
        )
        battn = np.concatenate(
            [
                b_attn[cols],
                b_attn[NX + g * M_LOC : NX + (g + 1) * M_LOC],
                b_attn[2 * NX + g * M_LOC : 2 * NX + (g + 1) * M_LOC],
            ]
        )
        in_maps.append(
            {
                "xT": np.ascontiguousarray(x[b].T),
                "wqkv": np.ascontiguousarray(wqkv),
                "wproj": np.ascontiguousarray(w_proj[g * M_LOC : (g + 1) * M_LOC, :]),
                "battn": np.ascontiguousarray(battn),
            }
        )

    res = run_bass_kernel_spmd(nc, in_maps, core_ids=list(range(8))).results

    out = np.empty((B, S, NX), np.float32)
    present = np.empty((2, B, H, S, D), np.float32)
    for c in range(8):
        b, g = c // 2, c % 2
        hs = slice(g * HPC, (g + 1) * HPC)
        present[0, b, hs] = (
            res[c]["kT_out"].reshape(HPC, D, S).transpose(0, 2, 1)
        )
        present[1, b, hs] = res[c]["v_out"].reshape(S, HPC, D).transpose(1, 0, 2)
    for b in range(B):
        out[b] = res[2 * b]["out_part"] + res[2 * b + 1]["out_part"] + b_proj
    return out, present


# revision 18
# speedup vs baseline: 75.0997x; 75.0997x over previous
"""Trainium2 Bass kernel for nn_Attention (B=4,S=1024,NX=1024,H=16).

Sharding (8 cores): data-parallel over batch (4) x tensor-parallel over
head-groups (2 groups of 8 heads), Megatron-style. Each core computes
QKV projection for its 8 heads on its batch, causal attention, and the
partial c_proj contribution; the 2-way all-reduce after c_proj is done
as a pair-sum during the host-side gather (output of each core is a
distinct partial), which is mathematically the Megatron all-reduce.

Device kernel (per core, fully unrolled Tile program):
  - inputs: xT = x[b].T [NX,S], wqkv [NX,1536] (q|k|v cols of this
    head group), wproj [512,NX] (rows of this head group), battn [1536]
  - qT,kT ([m,s] layout) via matmul(lhsT=wqkv, rhs=xT) + bias
  - v ([s,m] layout) via matmul(lhsT=xT, rhs=w_v) + bias (broadcast)
  - scores transposed S^T[sk,sq] = kT.T @ qT per head, exp on ScalarE
    (scale=1/8 folded in, no max-subtraction needed: |logits|<~6),
    causal handled structurally: fully-masked blocks never computed,
    diagonal blocks multiplied by a triangular 0/1 mask built on-device
  - av: aT[d,sq] = v65.T @ P^T with a ones-column in v65 producing the
    softmax denominator as row 64 of the PSUM tile (partition reduce
    done by the PE); normalize with reciprocal broadcast
  - partial out = aT.T @ wproj accumulated in PSUM, DMA'd straight out
All matmuls run as float32r (full PE rate at N=512, near-fp32 accuracy).
"""

import sys

sys.path.insert(0, "/opt/trn_rl_repo")

import numpy as np

import concourse.bass as bass
import concourse.mybir as mybir
import concourse.tile as tile
from concourse import bacc

B, S, NX, H = 4, 1024, 1024, 16
D = NX // H            # 64
HPC = 8                # heads per core (2 groups)
M_LOC = HPC * D        # 512 columns of q/k/v per core
P = 128
NK = NX // P           # 8 contraction chunks
SC = S // P            # 8 sequence chunks of 128
F32 = mybir.dt.float32
F32R = mybir.dt.float32r
AF = mybir.ActivationFunctionType
ALU = mybir.AluOpType


def _r(ap):
    """float32r view of an fp32 AP (full-rate PE matmul)."""
    return ap


def build_nc():
    nc = bacc.Bacc("TRN2", target_bir_lowering=False, debug=False, num_devices=8)
    # f32r = fp32 storage, reduced-precision full-rate PE matmul. Inputs
    # feeding matmuls are declared f32r end-to-end (DMA preserves dtype,
    # satisfying the BIR verifier's rounding rule); np side sees float32.
    xT = nc.dram_tensor("xT", [NX, S], F32R, kind="ExternalInput")
    wqkv = nc.dram_tensor("wqkv", [NX, 3 * M_LOC], F32R, kind="ExternalInput")
    wproj = nc.dram_tensor("wproj", [M_LOC, NX], F32R, kind="ExternalInput")
    battn = nc.dram_tensor("battn", [3 * M_LOC], F32, kind="ExternalInput")
    out_part = nc.dram_tensor("out_part", [S, NX], F32, kind="ExternalOutput")
    kT_out = nc.dram_tensor("kT_out", [M_LOC, S], F32R, kind="ExternalOutput")
    v_out = nc.dram_tensor("v_out", [S, M_LOC], F32R, kind="ExternalOutput")

    xT_t = xT.rearrange("(nk p) s -> p nk s", p=P)
    wqkv_t = wqkv.rearrange("(nk p) m -> p nk m", p=P)
    wproj_t = wproj.rearrange("(mk p) n -> p mk n", p=P)
    battn_t = battn.rearrange("(mc p) -> p mc", p=P)
    out_t = out_part.rearrange("(sc p) n -> p sc n", p=P)
    kT_t = kT_out.rearrange("(kc p) s -> p kc s", p=P)
    v_t = v_out.rearrange("(sc p) m -> p sc m", p=P)

    with tile.TileContext(nc) as tc:
        with (
            tc.tile_pool(name="const", bufs=1) as const,
            tc.tile_pool(name="wq", bufs=3) as wq_pool,
            tc.tile_pool(name="vb", bufs=3) as vb_pool,
            tc.tile_pool(name="pt", bufs=2) as pt_pool,
            tc.tile_pool(name="small", bufs=3) as small,
            tc.tile_pool(name="ps_mm", bufs=2, space="PSUM") as ps_mm,
            tc.tile_pool(name="ps_pt", bufs=2, space="PSUM") as ps_pt,
            tc.tile_pool(name="ps_a", bufs=2, space="PSUM") as ps_a,
        ):
            # ---- resident tiles ----
            xT_sb = const.tile([P, NK, S], F32R)         # x[b].T
            qkT_sb = const.tile([P, 8, S], F32R)         # qT (mc 0-3) | kT (mc 4-7)
            wv_sb = const.tile([P, NK, M_LOC], F32R)     # v columns of wqkv
            wproj_sb = const.tile([P, 4, NX], F32R)
            v65_sb = const.tile([P, SC, HPC, D + 1], F32R)  # v + ones col
            aT_sb = const.tile([P, 4, S], F32R)          # normalized attn out ^T
            battn_sb = const.tile([P, 12], F32)
            bv_row = const.tile([1, M_LOC], F32)
            bv_rep = const.tile([P, M_LOC], F32)
            tri_sb = const.tile([P, P], F32)             # keep where col>=row

            nc.sync.dma_start(xT_sb[:], xT_t)
            nc.sync.dma_start(wv_sb[:], wqkv_t[:, :, 2 * M_LOC : 3 * M_LOC])
            nc.sync.dma_start(wproj_sb[:], wproj_t)
            nc.sync.dma_start(battn_sb[:], battn_t)
            nc.sync.dma_start(
                bv_row[:], battn[2 * M_LOC : 3 * M_LOC].rearrange("(a m) -> a m", a=1)
            )
            nc.gpsimd.partition_broadcast(bv_rep[:], bv_row[:])

            nc.gpsimd.memset(tri_sb[:], 1.0)
            nc.gpsimd.affine_select(
                out=tri_sb[:],
                in_=tri_sb[:],
                pattern=[[1, P]],
                compare_op=ALU.is_ge,
                fill=0.0,
                base=0,
                channel_multiplier=-1,
            )


            # ---- phase B1: qT / kT  (out[m_chunk, s] = wqkv.T @ xT) ----
            for mc in range(8):
                wq_sb = wq_pool.tile([P, NK, P], F32R, tag="wq")
                nc.sync.dma_start(wq_sb[:], wqkv_t[:, :, mc * P : (mc + 1) * P])
                for sc in range(2):
                    ps = ps_mm.tile([P, 512], F32, tag="mm")
                    for nk in range(NK):
                        nc.tensor.matmul(
                            ps[:],
                            _r(wq_sb[:, nk, :]),
                            _r(xT_sb[:, nk, sc * 512 : (sc + 1) * 512]),
                            start=(nk == 0),
                            stop=(nk == NK - 1),
                        )
                    nc.vector.tensor_scalar_add(
                        qkT_sb[:, mc, sc * 512 : (sc + 1) * 512],
                        ps[:],
                        battn_sb[:, mc : mc + 1],
                    )
            for kc in range(4):
                nc.sync.dma_start(kT_t[:, kc], qkT_sb[:, 4 + kc, :])

            # ---- phase B2: v natural (out[s_chunk, m] = xT.T @ w_v) ----
            for sc in range(SC):
                ps = ps_mm.tile([P, 512], F32, tag="mm")
                for nk in range(NK):
                    nc.tensor.matmul(
                        ps[:],
                        _r(xT_sb[:, nk, sc * P : (sc + 1) * P]),
                        _r(wv_sb[:, nk, :]),
                        start=(nk == 0),
                        stop=(nk == NK - 1),
                    )
                vb_sb = vb_pool.tile([P, M_LOC], F32R, tag="vb")
                nc.vector.tensor_tensor(vb_sb[:], ps[:], bv_rep[:], ALU.add)
                nc.sync.dma_start(v_t[:, sc], vb_sb[:])
                for h in range(HPC):
                    nc.vector.tensor_copy(
                        out=v65_sb[:, sc, h, 0:D], in_=vb_sb[:, h * D : (h + 1) * D]
                    )
                # ones column (softmax denominator row): 0*in + 1 via ACT
                nc.scalar.activation(
                    v65_sb[:, sc, :, D], vb_sb[:, 0:HPC], AF.Copy, scale=0.0, bias=1.0
                )

            # ---- phase C: attention per (sq half, head) ----
            for sqh in range(2):
                L = (sqh + 1) * 512          # causal kv length for this q half
                nsk = L // P
                for h in range(HPC):
                    po = (h % 2) * 64
                    qmc = h // 2
                    kmc = 4 + h // 2
                    pt_sb = pt_pool.tile([P, SC, 512], F32R, tag="pt")
                    for skc in range(nsk):
                        j_tri = skc - sqh * 4  # diag block index within this half
                        j0 = max(0, j_tri)
                        ncols = 512 - j0 * P
                        ps = ps_pt.tile([P, 512], F32, tag="pt")
                        nc.tensor.matmul(
                            ps[:, j0 * P : 512],
                            _r(qkT_sb[po : po + 64, kmc, skc * P : (skc + 1) * P]),
                            _r(
                                qkT_sb[
                                    po : po + 64,
                                    qmc,
                                    sqh * 512 + j0 * P : (sqh + 1) * 512,
                                ]
                            ),
                            start=True,
                            stop=True,
                        )
                        nc.scalar.activation(
                            pt_sb[:, skc, j0 * P : 512],
                            ps[:, j0 * P : 512],
                            AF.Exp,
                            scale=float(1.0 / np.sqrt(D)),
                        )
                        if j_tri >= 0:
                            nc.vector.tensor_tensor(
                                pt_sb[:, skc, j_tri * P : (j_tri + 1) * P],
                                pt_sb[:, skc, j_tri * P : (j_tri + 1) * P],
                                tri_sb[:],
                                ALU.mult,
                            )
                    # av: aT[d,sq] (+ denominator in row 64 via ones column);
                    # each k chunk only touches its causally-valid sq columns
                    pa = ps_a.tile([D + 1, 512], F32, tag="pa")
                    for skc in range(nsk):
                        j0 = max(0, skc - sqh * 4)
                        nc.tensor.matmul(
                            pa[:, j0 * P : 512],
                            _r(v65_sb[:, skc, h, :]),
                            _r(pt_sb[:, skc, j0 * P : 512]),
                            start=(skc == 0),
                            stop=(skc == nsk - 1),
                        )
                    rinv = small.tile([1, 512], F32, tag="rinv")
                    nc.vector.reciprocal(rinv[:], pa[D : D + 1, :])
                    rrep = small.tile([D, 512], F32, tag="rrep")
                    nc.gpsimd.partition_broadcast(rrep[:], rinv[:])
                    a64 = small.tile([D, 512], F32R, tag="a64")
                    nc.vector.tensor_tensor(a64[:], pa[0:D, :], rrep[:], ALU.mult)
                    # place head rows at partitions po..po+64 of aT chunk h//2
                    nc.sync.dma_start(
                        aT_sb[po : po + 64, h // 2, sqh * 512 : (sqh + 1) * 512],
                        a64[:],
                    )

            # ---- phase D: partial c_proj ----
            for sc in range(SC):
                for nch in range(2):
                    ps = ps_mm.tile([P, 512], F32, tag="mm")
                    for mk in range(4):
                        nc.tensor.matmul(
                            ps[:],
                            _r(aT_sb[:, mk, sc * P : (sc + 1) * P]),
                            _r(wproj_sb[:, mk, nch * 512 : (nch + 1) * 512]),
                            start=(mk == 0),
                            stop=(mk == 3),
                        )
                    o_sb = vb_pool.tile([P, 512], F32, tag="osb")
                    nc.vector.tensor_copy(out=o_sb[:], in_=ps[:])
                    nc.sync.dma_start(out_t[:, sc, nch * 512 : (nch + 1) * 512], o_sb[:])

    nc.compile()
    return nc


_NC = None


def make_in_maps(inputs):
    x = np.asarray(inputs["x"], dtype=np.float32)
    w_attn = np.asarray(inputs["w_attn"], dtype=np.float32)
    b_attn = np.asarray(inputs["b_attn"], dtype=np.float32)

    in_maps = []
    for c in range(8):
        b, g = c // 2, c % 2
        cols = slice(g * M_LOC, (g + 1) * M_LOC)
        wqkv = np.concatenate(
            [
                w_attn[:, cols],
                w_attn[:, NX + g * M_LOC : NX + (g + 1) * M_LOC],
                w_attn[:, 2 * NX + g * M_LOC : 2 * NX + (g + 1) * M_LOC],
            ],
            axis=1,
        )
        battn = np.concatenate(
            [
                b_attn[cols],
                b_attn[NX + g * M_LOC : NX + (g + 1) * M_LOC],
                b_attn[2 * NX + g * M_LOC : 2 * NX + (g + 1) * M_LOC],
            ]
        )
        in_maps.append(
            {
                "xT": np.ascontiguousarray(x[b].T),
                "wqkv": np.ascontiguousarray(wqkv),
                "wproj": np.ascontiguousarray(
                    np.asarray(inputs["w_proj"], dtype=np.float32)[
                        g * M_LOC : (g + 1) * M_LOC, :
                    ]
                ),
                "battn": np.ascontiguousarray(battn),
            }
        )
    return in_maps


def kernel(**inputs):
    global _NC
    from concourse.bass_utils import run_bass_kernel_spmd

    b_proj = np.asarray(inputs["b_proj"], dtype=np.float32)
    if _NC is None:
        _NC = build_nc()
    nc = _NC
    in_maps = make_in_maps(inputs)

    res = run_bass_kernel_spmd(nc, in_maps, core_ids=list(range(8))).results

    out = np.empty((B, S, NX), np.float32)
    present = np.empty((2, B, H, S, D), np.float32)
    for c in range(8):
        b, g = c // 2, c % 2
        hs = slice(g * HPC, (g + 1) * HPC)
        present[0, b, hs] = (
            res[c]["kT_out"].reshape(HPC, D, S).transpose(0, 2, 1)
        )
        present[1, b, hs] = res[c]["v_out"].reshape(S, HPC, D).transpose(1, 0, 2)
    for b in range(B):
        out[b] = res[2 * b]["out_part"] + res[2 * b + 1]["out_part"] + b_proj
    return out, present


# revision 28
# speedup vs baseline: 45184.6265x; 601.6622x over previous
"""Trainium2 Bass kernel for nn_Attention (B=4,S=1024,NX=1024,H=16).

Sharding (8 cores): data-parallel over batch (4) x tensor-parallel over
head-groups (2 groups of 8 heads), Megatron-style. Each core computes
QKV projection for its 8 heads on its batch, causal attention, and the
partial c_proj contribution; the 2-way all-reduce after c_proj is done
as a pair-sum during the host-side gather (output of each core is a
distinct partial), which is mathematically the Megatron all-reduce.

Device kernel (per core, fully unrolled Tile program):
  - inputs: xT = x[b].T [NX,S], wqkv [NX,1536] (q|k|v cols of this
    head group), wproj [512,NX] (rows of this head group), battn [1536]
  - qT,kT ([m,s] layout) via matmul(lhsT=wqkv, rhs=xT) + bias
  - v ([s,m] layout) via matmul(lhsT=xT, rhs=w_v) + bias (broadcast)
  - scores transposed S^T[sk,sq] = kT.T @ qT per head, exp on ScalarE
    (scale=1/8 folded in, no max-subtraction needed: |logits|<~6),
    causal handled structurally: fully-masked blocks never computed,
    diagonal blocks multiplied by a triangular 0/1 mask built on-device
  - av: aT[d,sq] = v65.T @ P^T with a ones-column in v65 producing the
    softmax denominator as row 64 of the PSUM tile (partition reduce
    done by the PE); normalize with reciprocal broadcast
  - partial out = aT.T @ wproj accumulated in PSUM, DMA'd straight out
All matmuls run as float32r (full PE rate at N=512, near-fp32 accuracy).
"""

import sys

sys.path.insert(0, "/opt/trn_rl_repo")

import numpy as np

import concourse.bass as bass
import concourse.mybir as mybir
import concourse.tile as tile
from concourse import bacc

B, S, NX, H = 4, 1024, 1024, 16
D = NX // H            # 64
HPC = 8                # heads per core (2 groups)
M_LOC = HPC * D        # 512 columns of q/k/v per core
P = 128
NK = NX // P           # 8 contraction chunks
SC = S // P            # 8 sequence chunks of 128
F32 = mybir.dt.float32
F32R = mybir.dt.float32r
AF = mybir.ActivationFunctionType
ALU = mybir.AluOpType


def _r(ap):
    """float32r view of an fp32 AP (full-rate PE matmul)."""
    return ap


def build_nc():
    nc = bacc.Bacc("TRN2", target_bir_lowering=False, debug=False, num_devices=8)
    # f32r = fp32 storage, reduced-precision full-rate PE matmul. Inputs
    # feeding matmuls are declared f32r end-to-end (DMA preserves dtype,
    # satisfying the BIR verifier's rounding rule); np side sees float32.
    xT = nc.dram_tensor("xT", [NX, S], F32R, kind="ExternalInput")
    wqkv = nc.dram_tensor("wqkv", [NX, 3 * M_LOC], F32R, kind="ExternalInput")
    wproj = nc.dram_tensor("wproj", [M_LOC, NX], F32R, kind="ExternalInput")
    battn = nc.dram_tensor("battn", [3 * M_LOC], F32, kind="ExternalInput")
    out_part = nc.dram_tensor("out_part", [S, NX], F32, kind="ExternalOutput")
    kT_out = nc.dram_tensor("kT_out", [M_LOC, S], F32R, kind="ExternalOutput")
    v_out = nc.dram_tensor("v_out", [S, M_LOC], F32R, kind="ExternalOutput")

    xT_t = xT.rearrange("(nk p) s -> p nk s", p=P)
    wqkv_t = wqkv.rearrange("(nk p) m -> p nk m", p=P)
    wproj_t = wproj.rearrange("(mk p) n -> p mk n", p=P)
    battn_t = battn.rearrange("(mc p) -> p mc", p=P)
    out_t = out_part.rearrange("(sc p) n -> p sc n", p=P)
    kT_t = kT_out.rearrange("(kc p) s -> p kc s", p=P)
    v_t = v_out.rearrange("(sc p) m -> p sc m", p=P)

    with tile.TileContext(nc) as tc:
        with (
            tc.tile_pool(name="const", bufs=1) as const,
            tc.tile_pool(name="wq", bufs=3) as wq_pool,
            tc.tile_pool(name="vb", bufs=3) as vb_pool,
            tc.tile_pool(name="pt", bufs=2) as pt_pool,
            tc.tile_pool(name="small", bufs=3) as small,
            tc.tile_pool(name="ps_mm", bufs=3, space="PSUM") as ps_mm,
            tc.tile_pool(name="ps_pt", bufs=2, space="PSUM") as ps_pt,
            tc.tile_pool(name="ps_a", bufs=2, space="PSUM") as ps_a,
        ):
            # ---- resident tiles ----
            xT_sb = const.tile([P, NK, S], F32R)         # x[b].T
            qkT_sb = const.tile([P, 8, S], F32R)         # qT (mc 0-3) | kT (mc 4-7)
            wv_sb = const.tile([P, NK, M_LOC], F32R)     # v columns of wqkv
            wproj_sb = const.tile([P, 4, NX], F32R)
            v65_sb = const.tile([P, SC, HPC, D + 1], F32R)  # v + ones col
            aT_sb = const.tile([P, 4, S], F32R)          # normalized attn out ^T
            battn_sb = const.tile([P, 12], F32)
            bv_row = const.tile([1, M_LOC], F32)
            bv_rep = const.tile([P, M_LOC], F32)
            tri_sb = const.tile([P, P], F32)             # keep where col>=row

            # first qk weight chunk ahead of everything, then xT per chunk so
            # the first matmuls start as soon as chunk 0 lands
            wq_first = wq_pool.tile([P, NK, P], F32R, tag="wq")
            nc.sync.dma_start(wq_first[:], wqkv_t[:, :, 0:P])
            for nk in range(NK):
                nc.sync.dma_start(xT_sb[:, nk], xT_t[:, nk])
            nc.sync.dma_start(battn_sb[:], battn_t)
            nc.sync.dma_start(
                bv_row[:], battn[2 * M_LOC : 3 * M_LOC].rearrange("(a m) -> a m", a=1)
            )
            nc.gpsimd.partition_broadcast(bv_rep[:], bv_row[:])

            nc.gpsimd.memset(tri_sb[:], 1.0)
            nc.gpsimd.affine_select(
                out=tri_sb[:],
                in_=tri_sb[:],
                pattern=[[1, P]],
                compare_op=ALU.is_ge,
                fill=0.0,
                base=0,
                channel_multiplier=-1,
            )


            # ---- phase B1: qT / kT  (out[m_chunk, s] = wqkv.T @ xT) ----
            for mc in range(8):
                if mc == 0:
                    wq_sb = wq_first
                else:
                    wq_sb = wq_pool.tile([P, NK, P], F32R, tag="wq")
                    nc.sync.dma_start(wq_sb[:], wqkv_t[:, :, mc * P : (mc + 1) * P])
                for sc in range(2):
                    ps = ps_mm.tile([P, 512], F32, tag="mm")
                    for nk in range(NK):
                        nc.tensor.matmul(
                            ps[:],
                            _r(wq_sb[:, nk, :]),
                            _r(xT_sb[:, nk, sc * 512 : (sc + 1) * 512]),
                            start=(nk == 0),
                            stop=(nk == NK - 1),
                        )
                    nc.any.tensor_scalar_add(
                        qkT_sb[:, mc, sc * 512 : (sc + 1) * 512],
                        ps[:],
                        battn_sb[:, mc : mc + 1],
                    )
            for kc in range(4):
                nc.sync.dma_start(kT_t[:, kc], qkT_sb[:, 4 + kc, :])

            # ---- phase B2: v natural (out[s_chunk, m] = xT.T @ w_v) ----
            nc.sync.dma_start(wv_sb[:], wqkv_t[:, :, 2 * M_LOC : 3 * M_LOC])
            for sc in range(SC):
                ps = ps_mm.tile([P, 512], F32, tag="mm")
                for nk in range(NK):
                    nc.tensor.matmul(
                        ps[:],
                        _r(xT_sb[:, nk, sc * P : (sc + 1) * P]),
                        _r(wv_sb[:, nk, :]),
                        start=(nk == 0),
                        stop=(nk == NK - 1),
                    )
                vb_sb = vb_pool.tile([P, M_LOC], F32R, tag="vb")
                nc.vector.tensor_tensor(vb_sb[:], ps[:], bv_rep[:], ALU.add)
                nc.sync.dma_start(v_t[:, sc], vb_sb[:])
                for h in range(HPC):
                    nc.vector.tensor_copy(
                        out=v65_sb[:, sc, h, 0:D], in_=vb_sb[:, h * D : (h + 1) * D]
                    )
                # ones column (softmax denominator row): 0*in + 1 via ACT
                nc.scalar.activation(
                    v65_sb[:, sc, :, D], vb_sb[:, 0:HPC], AF.Copy, scale=0.0, bias=1.0
                )

            # ---- phase C: attention per (sq half, head) ----
            # prefetch c_proj weights; lands well before phase D needs them
            nc.sync.dma_start(wproj_sb[:], wproj_t)
            for sqh in range(2):
                L = (sqh + 1) * 512          # causal kv length for this q half
                nsk = L // P
                for h in range(HPC):
                    po = (h % 2) * 64
                    qmc = h // 2
                    kmc = 4 + h // 2
                    pt_sb = pt_pool.tile([P, SC, 512], F32R, tag="pt")
                    for skc in range(nsk):
                        j_tri = skc - sqh * 4  # diag block index within this half
                        j0 = max(0, j_tri)
                        ncols = 512 - j0 * P
                        ps = ps_pt.tile([P, 512], F32, tag="pt")
                        nc.tensor.matmul(
                            ps[:, j0 * P : 512],
                            _r(qkT_sb[po : po + 64, kmc, skc * P : (skc + 1) * P]),
                            _r(
                                qkT_sb[
                                    po : po + 64,
                                    qmc,
                                    sqh * 512 + j0 * P : (sqh + 1) * 512,
                                ]
                            ),
                            start=True,
                            stop=True,
                        )
                        nc.scalar.activation(
                            pt_sb[:, skc, j0 * P : 512],
                            ps[:, j0 * P : 512],
                            AF.Exp,
                            scale=float(1.0 / np.sqrt(D)),
                        )
                        if j_tri >= 0:
                            nc.vector.tensor_tensor(
                                pt_sb[:, skc, j_tri * P : (j_tri + 1) * P],
                                pt_sb[:, skc, j_tri * P : (j_tri + 1) * P],
                                tri_sb[:],
                                ALU.mult,
                            )
                    # av: aT[d,sq] (+ denominator in row 64 via ones column);
                    # each k chunk only touches its causally-valid sq columns
                    pa = ps_a.tile([D + 1, 512], F32, tag="pa")
                    for skc in range(nsk):
                        j0 = max(0, skc - sqh * 4)
                        nc.tensor.matmul(
                            pa[:, j0 * P : 512],
                            _r(v65_sb[:, skc, h, :]),
                            _r(pt_sb[:, skc, j0 * P : 512]),
                            start=(skc == 0),
                            stop=(skc == nsk - 1),
                        )
                    rinv = small.tile([1, 512], F32, tag="rinv")
                    nc.vector.reciprocal(rinv[:], pa[D : D + 1, :])
                    rrep = small.tile([D, 512], F32, tag="rrep")
                    nc.gpsimd.partition_broadcast(rrep[:], rinv[:])
                    a64 = small.tile([D, 512], F32R, tag="a64")
                    nc.vector.tensor_tensor(a64[:], pa[0:D, :], rrep[:], ALU.mult)
                    # place head rows at partitions po..po+64 of aT chunk h//2
                    nc.sync.dma_start(
                        aT_sb[po : po + 64, h // 2, sqh * 512 : (sqh + 1) * 512],
                        a64[:],
                    )

            # ---- phase D: partial c_proj ----
            for sc in range(SC):
                for nch in range(2):
                    ps = ps_mm.tile([P, 512], F32, tag="mm")
                    for mk in range(4):
                        nc.tensor.matmul(
                            ps[:],
                            _r(aT_sb[:, mk, sc * P : (sc + 1) * P]),
                            _r(wproj_sb[:, mk, nch * 512 : (nch + 1) * 512]),
                            start=(mk == 0),
                            stop=(mk == 3),
                        )
                    o_sb = vb_pool.tile([P, 512], F32, tag="osb")
                    nc.any.tensor_copy(out=o_sb[:], in_=ps[:])
                    nc.sync.dma_start(out_t[:, sc, nch * 512 : (nch + 1) * 512], o_sb[:])

    nc.compile()
    return nc


_NC = None


def make_in_maps(inputs):
    x = np.asarray(inputs["x"], dtype=np.float32)
    w_attn = np.asarray(inputs["w_attn"], dtype=np.float32)
    b_attn = np.asarray(inputs["b_attn"], dtype=np.float32)

    in_maps = []
    for c in range(8):
        b, g = c // 2, c % 2
        cols = slice(g * M_LOC, (g + 1) * M_LOC)
        wqkv = np.concatenate(
            [
                w_attn[:, cols],
                w_attn[:, NX + g * M_LOC : NX + (g + 1) * M_LOC],
                w_attn[:, 2 * NX + g * M_LOC : 2 * NX + (g + 1) * M_LOC],
            ],
            axis=1,
        )
        battn = np.concatenate(
            [
                b_attn[cols],
                b_attn[NX + g * M_LOC : NX + (g + 1) * M_LOC],
                b_attn[2 * NX + g * M_LOC : 2 * NX + (g + 1) * M_LOC],
            ]
        )
        in_maps.append(
            {
                "xT": np.ascontiguousarray(x[b].T),
                "wqkv": np.ascontiguousarray(wqkv),
                "wproj": np.ascontiguousarray(
                    np.asarray(inputs["w_proj"], dtype=np.float32)[
                        g * M_LOC : (g + 1) * M_LOC, :
                    ]
                ),
                "battn": np.ascontiguousarray(battn),
            }
        )
    return in_maps


def kernel(**inputs):
    global _NC
    from concourse.bass_utils import run_bass_kernel_spmd

    b_proj = np.asarray(inputs["b_proj"], dtype=np.float32)
    if _NC is None:
        _NC = build_nc()
    nc = _NC
    in_maps = make_in_maps(inputs)

    res = run_bass_kernel_spmd(nc, in_maps, core_ids=list(range(8))).results

    out = np.empty((B, S, NX), np.float32)
    present = np.empty((2, B, H, S, D), np.float32)
    for c in range(8):
        b, g = c // 2, c % 2
        hs = slice(g * HPC, (g + 1) * HPC)
        present[0, b, hs] = (
            res[c]["kT_out"].reshape(HPC, D, S).transpose(0, 2, 1)
        )
        present[1, b, hs] = res[c]["v_out"].reshape(S, HPC, D).transpose(1, 0, 2)
    for b in range(B):
        out[b] = res[2 * b]["out_part"] + res[2 * b + 1]["out_part"] + b_proj
    return out, present


# revision 29
# speedup vs baseline: 46993.7346x; 1.0400x over previous
"""Trainium2 Bass kernel for nn_Attention (B=4,S=1024,NX=1024,H=16).

Sharding (8 cores): data-parallel over batch (4) x tensor-parallel over
head-groups (2 groups of 8 heads), Megatron-style. Each core computes
QKV projection for its 8 heads on its batch, causal attention, and the
partial c_proj contribution; the 2-way all-reduce after c_proj is done
as a pair-sum during the host-side gather (output of each core is a
distinct partial), which is mathematically the Megatron all-reduce.

Device kernel (per core, fully unrolled Tile program):
  - inputs: xT = x[b].T [NX,S], wqkv [NX,1536] (q|k|v cols of this
    head group), wproj [512,NX] (rows of this head group), battn [1536]
  - qT,kT ([m,s] layout) via matmul(lhsT=wqkv, rhs=xT) + bias
  - v ([s,m] layout) via matmul(lhsT=xT, rhs=w_v) + bias (broadcast)
  - scores transposed S^T[sk,sq] = kT.T @ qT per head, exp on ScalarE
    (scale=1/8 folded in, no max-subtraction needed: |logits|<~6),
    causal handled structurally: fully-masked blocks never computed,
    diagonal blocks multiplied by a triangular 0/1 mask built on-device
  - av: aT[d,sq] = v65.T @ P^T with a ones-column in v65 producing the
    softmax denominator as row 64 of the PSUM tile (partition reduce
    done by the PE); normalize with reciprocal broadcast
  - partial out = aT.T @ wproj accumulated in PSUM, DMA'd straight out
All matmuls run as float32r (full PE rate at N=512, near-fp32 accuracy).
"""

import sys

sys.path.insert(0, "/opt/trn_rl_repo")

import numpy as np

import concourse.bass as bass
import concourse.mybir as mybir
import concourse.tile as tile
from concourse import bacc

B, S, NX, H = 4, 1024, 1024, 16
D = NX // H            # 64
HPC = 8                # heads per core (2 groups)
M_LOC = HPC * D        # 512 columns of q/k/v per core
P = 128
NK = NX // P           # 8 contraction chunks
SC = S // P            # 8 sequence chunks of 128
F32 = mybir.dt.float32
F32R = mybir.dt.float32r
AF = mybir.ActivationFunctionType
ALU = mybir.AluOpType


def _r(ap):
    """float32r view of an fp32 AP (full-rate PE matmul)."""
    return ap


def build_nc():
    nc = bacc.Bacc("TRN2", target_bir_lowering=False, debug=False, num_devices=8)
    # f32r = fp32 storage, reduced-precision full-rate PE matmul. Inputs
    # feeding matmuls are declared f32r end-to-end (DMA preserves dtype,
    # satisfying the BIR verifier's rounding rule); np side sees float32.
    xT = nc.dram_tensor("xT", [NX, S], F32R, kind="ExternalInput")
    wqkv = nc.dram_tensor("wqkv", [NX, 3 * M_LOC], F32R, kind="ExternalInput")
    wproj = nc.dram_tensor("wproj", [M_LOC, NX], F32R, kind="ExternalInput")
    battn = nc.dram_tensor("battn", [3 * M_LOC], F32, kind="ExternalInput")
    out_part = nc.dram_tensor("out_part", [S, NX], F32, kind="ExternalOutput")
    kT_out = nc.dram_tensor("kT_out", [M_LOC, S], F32R, kind="ExternalOutput")
    v_out = nc.dram_tensor("v_out", [S, M_LOC], F32R, kind="ExternalOutput")

    xT_t = xT.rearrange("(nk p) s -> p nk s", p=P)
    wqkv_t = wqkv.rearrange("(nk p) m -> p nk m", p=P)
    wproj_t = wproj.rearrange("(mk p) n -> p mk n", p=P)
    battn_t = battn.rearrange("(mc p) -> p mc", p=P)
    out_t = out_part.rearrange("(sc p) n -> p sc n", p=P)
    kT_t = kT_out.rearrange("(kc p) s -> p kc s", p=P)
    v_t = v_out.rearrange("(sc p) m -> p sc m", p=P)

    with tile.TileContext(nc) as tc:
        with (
            tc.tile_pool(name="const", bufs=1) as const,
            tc.tile_pool(name="wq", bufs=3) as wq_pool,
            tc.tile_pool(name="vb", bufs=3) as vb_pool,
            tc.tile_pool(name="pt", bufs=2) as pt_pool,
            tc.tile_pool(name="small", bufs=3) as small,
            tc.tile_pool(name="ps_mm", bufs=3, space="PSUM") as ps_mm,
            tc.tile_pool(name="ps_pt", bufs=2, space="PSUM") as ps_pt,
            tc.tile_pool(name="ps_a", bufs=2, space="PSUM") as ps_a,
        ):
            # ---- resident tiles ----
            xT_sb = const.tile([P, NK, S], F32R)         # x[b].T
            qkT_sb = const.tile([P, 8, S], F32R)         # qT (mc 0-3) | kT (mc 4-7)
            wv_sb = const.tile([P, NK, M_LOC], F32R)     # v columns of wqkv
            wproj_sb = const.tile([P, 4, NX], F32R)
            v65_sb = const.tile([P, SC, HPC, D + 1], F32R)  # v + ones col
            aT_sb = const.tile([P, 4, S], F32R)          # normalized attn out ^T
            battn_sb = const.tile([P, 12], F32)
            bv_row = const.tile([1, M_LOC], F32)
            bv_rep = const.tile([P, M_LOC], F32)
            tri_sb = const.tile([P, P], F32)             # keep where col>=row

            # first qk weight chunk ahead of everything, then xT per chunk so
            # the first matmuls start as soon as chunk 0 lands
            wq_first = wq_pool.tile([P, NK, P], F32R, tag="wq")
            nc.sync.dma_start(wq_first[:], wqkv_t[:, :, 0:P])
            for nk in range(NK):
                nc.sync.dma_start(xT_sb[:, nk], xT_t[:, nk])
            nc.sync.dma_start(battn_sb[:], battn_t)
            nc.sync.dma_start(
                bv_row[:], battn[2 * M_LOC : 3 * M_LOC].rearrange("(a m) -> a m", a=1)
            )
            nc.gpsimd.partition_broadcast(bv_rep[:], bv_row[:])

            nc.gpsimd.memset(tri_sb[:], 1.0)
            nc.gpsimd.affine_select(
                out=tri_sb[:],
                in_=tri_sb[:],
                pattern=[[1, P]],
                compare_op=ALU.is_ge,
                fill=0.0,
                base=0,
                channel_multiplier=-1,
            )


            # ---- phase B1: qT / kT  (out[m_chunk, s] = wqkv.T @ xT) ----
            for mc in range(8):
                if mc == 0:
                    wq_sb = wq_first
                else:
                    wq_sb = wq_pool.tile([P, NK, P], F32R, tag="wq")
                    nc.sync.dma_start(wq_sb[:], wqkv_t[:, :, mc * P : (mc + 1) * P])
                for sc in range(2):
                    ps = ps_mm.tile([P, 512], F32, tag="mm")
                    for nk in range(NK):
                        nc.tensor.matmul(
                            ps[:],
                            _r(wq_sb[:, nk, :]),
                            _r(xT_sb[:, nk, sc * 512 : (sc + 1) * 512]),
                            start=(nk == 0),
                            stop=(nk == NK - 1),
                        )
                    nc.any.tensor_scalar_add(
                        qkT_sb[:, mc, sc * 512 : (sc + 1) * 512],
                        ps[:],
                        battn_sb[:, mc : mc + 1],
                    )
            for kc in range(4):
                nc.sync.dma_start(kT_t[:, kc], qkT_sb[:, 4 + kc, :])

            # ---- phase B2: v natural (out[s_chunk, m] = xT.T @ w_v) ----
            nc.sync.dma_start(wv_sb[:], wqkv_t[:, :, 2 * M_LOC : 3 * M_LOC])
            for sc in range(SC):
                ps = ps_mm.tile([P, 512], F32, tag="mm")
                for nk in range(NK):
                    nc.tensor.matmul(
                        ps[:],
                        _r(xT_sb[:, nk, sc * P : (sc + 1) * P]),
                        _r(wv_sb[:, nk, :]),
                        start=(nk == 0),
                        stop=(nk == NK - 1),
                    )
                vb_sb = vb_pool.tile([P, M_LOC], F32R, tag="vb")
                nc.vector.tensor_tensor(vb_sb[:], ps[:], bv_rep[:], ALU.add)
                nc.sync.dma_start(v_t[:, sc], vb_sb[:])
                for h in range(HPC):
                    nc.vector.tensor_copy(
                        out=v65_sb[:, sc, h, 0:D], in_=vb_sb[:, h * D : (h + 1) * D]
                    )
                # ones column (softmax denominator row): 0*in + 1 via ACT
                nc.scalar.activation(
                    v65_sb[:, sc, :, D], vb_sb[:, 0:HPC], AF.Copy, scale=0.0, bias=1.0
                )

            # ---- phase C: attention per (sq half, head) ----
            # prefetch c_proj weights; lands well before phase D needs them
            nc.sync.dma_start(wproj_sb[:], wproj_t)
            for sqh in range(2):
                L = (sqh + 1) * 512          # causal kv length for this q half
                nsk = L // P
                for h in range(HPC):
                    po = (h % 2) * 64
                    qmc = h // 2
                    kmc = 4 + h // 2
                    pt_sb = pt_pool.tile([P, SC, 512], F32R, tag="pt")
                    for skc in range(nsk):
                        j_tri = skc - sqh * 4  # diag block index within this half
                        j0 = max(0, j_tri)
                        ncols = 512 - j0 * P
                        ps = ps_pt.tile([P, 512], F32, tag="pt")
                        nc.tensor.matmul(
                            ps[:, j0 * P : 512],
                            _r(qkT_sb[po : po + 64, kmc, skc * P : (skc + 1) * P]),
                            _r(
                                qkT_sb[
                                    po : po + 64,
                                    qmc,
                                    sqh * 512 + j0 * P : (sqh + 1) * 512,
                                ]
                            ),
                            start=True,
                            stop=True,
                        )
                        nc.scalar.activation(
                            pt_sb[:, skc, j0 * P : 512],
                            ps[:, j0 * P : 512],
                            AF.Exp,
                            scale=float(1.0 / np.sqrt(D)),
                        )
                        if j_tri >= 0:
                            nc.vector.tensor_tensor(
                                pt_sb[:, skc, j_tri * P : (j_tri + 1) * P],
                                pt_sb[:, skc, j_tri * P : (j_tri + 1) * P],
                                tri_sb[:],
                                ALU.mult,
                            )
                    # av: aT[d,sq] (+ denominator in row 64 via ones column);
                    # each k chunk only touches its causally-valid sq columns
                    pa = ps_a.tile([D + 1, 512], F32, tag="pa")
                    for skc in range(nsk):
                        j0 = max(0, skc - sqh * 4)
                        nc.tensor.matmul(
                            pa[:, j0 * P : 512],
                            _r(v65_sb[:, skc, h, :]),
                            _r(pt_sb[:, skc, j0 * P : 512]),
                            start=(skc == 0),
                            stop=(skc == nsk - 1),
                        )
                    rinv = small.tile([1, 512], F32, tag="rinv")
                    nc.vector.reciprocal(rinv[:], pa[D : D + 1, :])
                    rrep = small.tile([D, 512], F32, tag="rrep")
                    nc.gpsimd.partition_broadcast(rrep[:], rinv[:])
                    # normalize straight into aT (DVE supports differing
                    # in/out base partitions — verified on HW via reciprocal)
                    nc.vector.tensor_tensor(
                        aT_sb[po : po + 64, h // 2, sqh * 512 : (sqh + 1) * 512],
                        pa[0:D, :],
                        rrep[:],
                        ALU.mult,
                    )

            # ---- phase D: partial c_proj ----
            for sc in range(SC):
                for nch in range(2):
                    ps = ps_mm.tile([P, 512], F32, tag="mm")
                    for mk in range(4):
                        nc.tensor.matmul(
                            ps[:],
                            _r(aT_sb[:, mk, sc * P : (sc + 1) * P]),
                            _r(wproj_sb[:, mk, nch * 512 : (nch + 1) * 512]),
                            start=(mk == 0),
                            stop=(mk == 3),
                        )
                    o_sb = vb_pool.tile([P, 512], F32, tag="osb")
                    nc.any.tensor_copy(out=o_sb[:], in_=ps[:])
                    nc.sync.dma_start(out_t[:, sc, nch * 512 : (nch + 1) * 512], o_sb[:])

    nc.compile()
    return nc


_NC = None


def make_in_maps(inputs):
    x = np.asarray(inputs["x"], dtype=np.float32)
    w_attn = np.asarray(inputs["w_attn"], dtype=np.float32)
    b_attn = np.asarray(inputs["b_attn"], dtype=np.float32)

    in_maps = []
    for c in range(8):
        b, g = c // 2, c % 2
        cols = slice(g * M_LOC, (g + 1) * M_LOC)
        wqkv = np.concatenate(
            [
                w_attn[:, cols],
                w_attn[:, NX + g * M_LOC : NX + (g + 1) * M_LOC],
                w_attn[:, 2 * NX + g * M_LOC : 2 * NX + (g + 1) * M_LOC],
            ],
            axis=1,
        )
        battn = np.concatenate(
            [
                b_attn[cols],
                b_attn[NX + g * M_LOC : NX + (g + 1) * M_LOC],
                b_attn[2 * NX + g * M_LOC : 2 * NX + (g + 1) * M_LOC],
            ]
        )
        in_maps.append(
            {
                "xT": np.ascontiguousarray(x[b].T),
                "wqkv": np.ascontiguousarray(wqkv),
                "wproj": np.ascontiguousarray(
                    np.asarray(inputs["w_proj"], dtype=np.float32)[
                        g * M_LOC : (g + 1) * M_LOC, :
                    ]
                ),
                "battn": np.ascontiguousarray(battn),
            }
        )
    return in_maps


def kernel(**inputs):
    global _NC
    from concourse.bass_utils import run_bass_kernel_spmd

    b_proj = np.asarray(inputs["b_proj"], dtype=np.float32)
    if _NC is None:
        _NC = build_nc()
    nc = _NC
    in_maps = make_in_maps(inputs)

    res = run_bass_kernel_spmd(nc, in_maps, core_ids=list(range(8))).results

    out = np.empty((B, S, NX), np.float32)
    present = np.empty((2, B, H, S, D), np.float32)
    for c in range(8):
        b, g = c // 2, c % 2
        hs = slice(g * HPC, (g + 1) * HPC)
        present[0, b, hs] = (
            res[c]["kT_out"].reshape(HPC, D, S).transpose(0, 2, 1)
        )
        present[1, b, hs] = res[c]["v_out"].reshape(S, HPC, D).transpose(1, 0, 2)
    for b in range(B):
        out[b] = res[2 * b]["out_part"] + res[2 * b + 1]["out_part"] + b_proj
    return out, present


# revision 38
# speedup vs baseline: 48573.3559x; 1.0336x over previous
"""Trainium2 Bass kernel for nn_Attention (B=4,S=1024,NX=1024,H=16).

Sharding (8 cores): data-parallel over batch (4) x tensor-parallel over
head-groups (2 groups of 8 heads), Megatron-style. Each core computes
QKV projection for its 8 heads on its batch, causal attention, and the
partial c_proj contribution; the 2-way all-reduce after c_proj is done
as a pair-sum during the host-side gather (output of each core is a
distinct partial), which is mathematically the Megatron all-reduce.

Device kernel (per core, fully unrolled Tile program):
  - inputs: xT = x[b].T [NX,S], wqkv [NX,1536] (q|k|v cols of this
    head group), wproj [512,NX] (rows of this head group), battn [1536]
  - qT,kT ([m,s] layout) via matmul(lhsT=wqkv, rhs=xT) + bias
  - v ([s,m] layout) via matmul(lhsT=xT, rhs=w_v) + bias (broadcast)
  - scores transposed S^T[sk,sq] = kT.T @ qT per head, exp on ScalarE
    (scale=1/8 folded in, no max-subtraction needed: |logits|<~6),
    causal handled structurally: fully-masked blocks never computed,
    diagonal blocks multiplied by a triangular 0/1 mask built on-device
  - av: aT[d,sq] = v65.T @ P^T with a ones-column in v65 producing the
    softmax denominator as row 64 of the PSUM tile (partition reduce
    done by the PE); normalize with reciprocal broadcast
  - partial out = aT.T @ wproj accumulated in PSUM, DMA'd straight out
All matmuls run as float32r (full PE rate at N=512, near-fp32 accuracy).
"""

import sys

sys.path.insert(0, "/opt/trn_rl_repo")

import numpy as np

import concourse.bass as bass
import concourse.mybir as mybir
import concourse.tile as tile
from concourse import bacc

B, S, NX, H = 4, 1024, 1024, 16
D = NX // H            # 64
HPC = 8                # heads per core (2 groups)
M_LOC = HPC * D        # 512 columns of q/k/v per core
P = 128
NK = NX // P           # 8 contraction chunks
SC = S // P            # 8 sequence chunks of 128
F32 = mybir.dt.float32
F32R = mybir.dt.float32r
AF = mybir.ActivationFunctionType
ALU = mybir.AluOpType


def _r(ap):
    """float32r view of an fp32 AP (full-rate PE matmul)."""
    return ap


def build_nc():
    nc = bacc.Bacc("TRN2", target_bir_lowering=False, debug=False, num_devices=8)
    # f32r = fp32 storage, reduced-precision full-rate PE matmul. Inputs
    # feeding matmuls are declared f32r end-to-end (DMA preserves dtype,
    # satisfying the BIR verifier's rounding rule); np side sees float32.
    xT = nc.dram_tensor("xT", [NX, S], F32R, kind="ExternalInput")
    wqkv = nc.dram_tensor("wqkv", [NX, 3 * M_LOC], F32R, kind="ExternalInput")
    wproj = nc.dram_tensor("wproj", [M_LOC, NX], F32R, kind="ExternalInput")
    battn = nc.dram_tensor("battn", [3 * M_LOC], F32, kind="ExternalInput")
    out_part = nc.dram_tensor("out_part", [S, NX], F32, kind="ExternalOutput")
    kT_out = nc.dram_tensor("kT_out", [M_LOC, S], F32R, kind="ExternalOutput")
    v_out = nc.dram_tensor("v_out", [S, M_LOC], F32R, kind="ExternalOutput")

    xT_t = xT.rearrange("(nk p) s -> p nk s", p=P)
    wqkv_t = wqkv.rearrange("(nk p) m -> p nk m", p=P)
    wproj_t = wproj.rearrange("(mk p) n -> p mk n", p=P)
    battn_t = battn.rearrange("(mc p) -> p mc", p=P)
    out_t = out_part.rearrange("(sc p) n -> p sc n", p=P)
    kT_t = kT_out.rearrange("(kc p) s -> p kc s", p=P)
    v_t = v_out.rearrange("(sc p) m -> p sc m", p=P)

    with tile.TileContext(nc) as tc:
        with (
            tc.tile_pool(name="const", bufs=1) as const,
            tc.tile_pool(name="wq", bufs=3) as wq_pool,
            tc.tile_pool(name="vb", bufs=3) as vb_pool,
            tc.tile_pool(name="pt", bufs=2) as pt_pool,
            tc.tile_pool(name="small", bufs=3) as small,
            tc.tile_pool(name="ps_mm", bufs=3, space="PSUM") as ps_mm,
            tc.tile_pool(name="ps_pt", bufs=3, space="PSUM") as ps_pt,
            tc.tile_pool(name="ps_a", bufs=2, space="PSUM") as ps_a,
        ):
            # ---- resident tiles ----
            xT_sb = const.tile([P, NK, S], F32R)         # x[b].T
            qkT_sb = const.tile([P, 8, S], F32R)         # qT (mc 0-3) | kT (mc 4-7)
            wv_sb = const.tile([P, NK, M_LOC], F32R)     # v columns of wqkv
            wproj_sb = const.tile([P, 4, NX], F32R)
            v65_sb = const.tile([P, SC, HPC, D + 1], F32R)  # v + ones col
            aT_sb = const.tile([P, 4, S], F32R)          # normalized attn out ^T
            battn_sb = const.tile([P, 12], F32)
            bv_row = const.tile([1, M_LOC], F32)
            bv_rep = const.tile([P, M_LOC], F32)
            tri_sb = const.tile([P, P], F32)             # keep where col>=row

            # first qk weight chunk ahead of everything, then xT per chunk so
            # the first matmuls start as soon as chunk 0 lands
            wq_first = wq_pool.tile([P, NK, P], F32R, tag="wq")
            nc.sync.dma_start(wq_first[:], wqkv_t[:, :, 0:P])
            for nk in range(NK):
                nc.sync.dma_start(xT_sb[:, nk], xT_t[:, nk])
            nc.sync.dma_start(battn_sb[:], battn_t)
            nc.sync.dma_start(
                bv_row[:], battn[2 * M_LOC : 3 * M_LOC].rearrange("(a m) -> a m", a=1)
            )
            nc.gpsimd.partition_broadcast(bv_rep[:], bv_row[:])

            nc.gpsimd.memset(tri_sb[:], 1.0)
            nc.gpsimd.affine_select(
                out=tri_sb[:],
                in_=tri_sb[:],
                pattern=[[1, P]],
                compare_op=ALU.is_ge,
                fill=0.0,
                base=0,
                channel_multiplier=-1,
            )


            # ---- phase B1: qT / kT  (out[m_chunk, s] = wqkv.T @ xT) ----
            for mc in range(8):
                if mc == 0:
                    wq_sb = wq_first
                else:
                    wq_sb = wq_pool.tile([P, NK, P], F32R, tag="wq")
                    nc.sync.dma_start(wq_sb[:], wqkv_t[:, :, mc * P : (mc + 1) * P])
                for sc in range(2):
                    ps = ps_mm.tile([P, 512], F32, tag="mm")
                    for nk in range(NK):
                        nc.tensor.matmul(
                            ps[:],
                            _r(wq_sb[:, nk, :]),
                            _r(xT_sb[:, nk, sc * 512 : (sc + 1) * 512]),
                            start=(nk == 0),
                            stop=(nk == NK - 1),
                        )
                    nc.any.tensor_scalar_add(
                        qkT_sb[:, mc, sc * 512 : (sc + 1) * 512],
                        ps[:],
                        battn_sb[:, mc : mc + 1],
                    )
            for kc in range(4):
                nc.sync.dma_start(kT_t[:, kc], qkT_sb[:, 4 + kc, :])

            # ---- phase B2: v natural (out[s_chunk, m] = xT.T @ w_v) ----
            nc.sync.dma_start(wv_sb[:], wqkv_t[:, :, 2 * M_LOC : 3 * M_LOC])
            for sc in range(SC):
                ps = ps_mm.tile([P, 512], F32, tag="mm")
                for nk in range(NK):
                    nc.tensor.matmul(
                        ps[:],
                        _r(xT_sb[:, nk, sc * P : (sc + 1) * P]),
                        _r(wv_sb[:, nk, :]),
                        start=(nk == 0),
                        stop=(nk == NK - 1),
                    )
                vb_sb = vb_pool.tile([P, M_LOC], F32R, tag="vb")
                nc.vector.tensor_tensor(vb_sb[:], ps[:], bv_rep[:], ALU.add)
                nc.sync.dma_start(v_t[:, sc], vb_sb[:])
                for h in range(HPC):
                    nc.vector.tensor_copy(
                        out=v65_sb[:, sc, h, 0:D], in_=vb_sb[:, h * D : (h + 1) * D]
                    )
                # ones column (softmax denominator row): 0*in + 1 via ACT
                nc.scalar.activation(
                    v65_sb[:, sc, :, D], vb_sb[:, 0:HPC], AF.Copy, scale=0.0, bias=1.0
                )

            # ---- phase C: attention per (sq half, head) ----
            # prefetch c_proj weights; lands well before phase D needs them
            nc.sync.dma_start(wproj_sb[:], wproj_t)
            for sqh in range(2):
                L = (sqh + 1) * 512          # causal kv length for this q half
                nsk = L // P
                for h in range(HPC):
                    po = (h % 2) * 64
                    qmc = h // 2
                    kmc = 4 + h // 2
                    pt_sb = pt_pool.tile([P, SC, 512], F32R, tag="pt")
                    for skc in range(nsk):
                        j_tri = skc - sqh * 4  # diag block index within this half
                        j0 = max(0, j_tri)
                        ps = ps_pt.tile([P, 512], F32, tag="pt")
                        nc.tensor.matmul(
                            ps[:, j0 * P : 512],
                            _r(qkT_sb[po : po + 64, kmc, skc * P : (skc + 1) * P]),
                            _r(
                                qkT_sb[
                                    po : po + 64,
                                    qmc,
                                    sqh * 512 + j0 * P : (sqh + 1) * 512,
                                ]
                            ),
                            start=True,
                            stop=True,
                        )
                        nc.scalar.activation(
                            pt_sb[:, skc, j0 * P : 512],
                            ps[:, j0 * P : 512],
                            AF.Exp,
                            scale=float(1.0 / np.sqrt(D)),
                        )
                        if j_tri >= 0:
                            nc.vector.tensor_tensor(
                                pt_sb[:, skc, j_tri * P : (j_tri + 1) * P],
                                pt_sb[:, skc, j_tri * P : (j_tri + 1) * P],
                                tri_sb[:],
                                ALU.mult,
                            )
                    # av: aT[d,sq] (+ denominator in row 64 via ones column);
                    # each k chunk only touches its causally-valid sq columns
                    pa = ps_a.tile([D + 1, 512], F32, tag="pa")
                    for skc in range(nsk):
                        j0 = max(0, skc - sqh * 4)
                        nc.tensor.matmul(
                            pa[:, j0 * P : 512],
                            _r(v65_sb[:, skc, h, :]),
                            _r(pt_sb[:, skc, j0 * P : 512]),
                            start=(skc == 0),
                            stop=(skc == nsk - 1),
                        )
                    rinv = small.tile([1, 512], F32, tag="rinv")
                    nc.vector.reciprocal(rinv[:], pa[D : D + 1, :])
                    rrep = small.tile([D, 512], F32, tag="rrep")
                    nc.gpsimd.partition_broadcast(rrep[:], rinv[:])
                    # normalize straight into aT (DVE supports differing
                    # in/out base partitions — verified on HW via reciprocal)
                    nc.vector.tensor_tensor(
                        aT_sb[po : po + 64, h // 2, sqh * 512 : (sqh + 1) * 512],
                        pa[0:D, :],
                        rrep[:],
                        ALU.mult,
                    )

            # ---- phase D: partial c_proj ----
            for sc in range(SC):
                for nch in range(2):
                    ps = ps_mm.tile([P, 512], F32, tag="mm")
                    for mk in range(4):
                        nc.tensor.matmul(
                            ps[:],
                            _r(aT_sb[:, mk, sc * P : (sc + 1) * P]),
                            _r(wproj_sb[:, mk, nch * 512 : (nch + 1) * 512]),
                            start=(mk == 0),
                            stop=(mk == 3),
                        )
                    o_sb = vb_pool.tile([P, 512], F32, tag="osb")
                    nc.any.tensor_copy(out=o_sb[:], in_=ps[:])
                    nc.sync.dma_start(out_t[:, sc, nch * 512 : (nch + 1) * 512], o_sb[:])

    nc.compile()
    return nc


_NC = None


def make_in_maps(inputs):
    x = np.asarray(inputs["x"], dtype=np.float32)
    w_attn = np.asarray(inputs["w_attn"], dtype=np.float32)
    b_attn = np.asarray(inputs["b_attn"], dtype=np.float32)

    in_maps = []
    for c in range(8):
        b, g = c // 2, c % 2
        cols = slice(g * M_LOC, (g + 1) * M_LOC)
        wqkv = np.concatenate(
            [
                w_attn[:, cols],
                w_attn[:, NX + g * M_LOC : NX + (g + 1) * M_LOC],
                w_attn[:, 2 * NX + g * M_LOC : 2 * NX + (g + 1) * M_LOC],
            ],
            axis=1,
        )
        battn = np.concatenate(
            [
                b_attn[cols],
                b_attn[NX + g * M_LOC : NX + (g + 1) * M_LOC],
                b_attn[2 * NX + g * M_LOC : 2 * NX + (g + 1) * M_LOC],
            ]
        )
        in_maps.append(
            {
                "xT": np.ascontiguousarray(x[b].T),
                "wqkv": np.ascontiguousarray(wqkv),
                "wproj": np.ascontiguousarray(
                    np.asarray(inputs["w_proj"], dtype=np.float32)[
                        g * M_LOC : (g + 1) * M_LOC, :
                    ]
                ),
                "battn": np.ascontiguousarray(battn),
            }
        )
    return in_maps


def kernel(**inputs):
    global _NC
    from concourse.bass_utils import run_bass_kernel_spmd

    b_proj = np.asarray(inputs["b_proj"], dtype=np.float32)
    if _NC is None:
        _NC = build_nc()
    nc = _NC
    in_maps = make_in_maps(inputs)

    res = run_bass_kernel_spmd(nc, in_maps, core_ids=list(range(8))).results

    out = np.empty((B, S, NX), np.float32)
    present = np.empty((2, B, H, S, D), np.float32)
    for c in range(8):
        b, g = c // 2, c % 2
        hs = slice(g * HPC, (g + 1) * HPC)
        present[0, b, hs] = (
            res[c]["kT_out"].reshape(HPC, D, S).transpose(0, 2, 1)
        )
        present[1, b, hs] = res[c]["v_out"].reshape(S, HPC, D).transpose(1, 0, 2)
    for b in range(B):
        out[b] = res[2 * b]["out_part"] + res[2 * b + 1]["out_part"] + b_proj
    return out, present
